# revision 4
# baseline (speedup 1.0000x reference)
"""Multi-head causal attention with RoPE on 8 Trainium2 NeuronCores.

Sharding: 2 batches x 4 head-groups. Core c owns batch c//4 and heads
4*(c%4)..4*(c%4)+3 (a 256-wide slice of D_OUT), processed as two local
head-pairs hp in {0,1}. Each core computes Q/K/V projections for its
slice (column-sliced Wq/Wk/Wv), RoPE, causal attention, and a row-sliced
out-projection partial for its batch. The 4 partials per batch are
summed on the host (the all-reduce of the row-parallel out projection)
and the bias added once.

Data path is bf16 (weights, x, rope tables, K/V/Q tiles, probabilities,
output partials); accumulation stays f32 in PSUM. vs the previous
head-only sharding this cuts per-core HBM traffic 4x (x read: 16->4MB,
out write: 16->4MB).

Emission is software-pipelined for the in-order PE: the next window's
projection matmuls are interleaved between exp and ctx inside the
attention loop (covering ACT latency), and each window's out-projection
can be delayed one window and interleaved the same way.
"""

import sys

sys.path.insert(0, "/opt/trn_rl_repo")

from contextlib import ExitStack

import numpy as np

import concourse.bass as bass
import concourse.tile as tile
from concourse import bacc, mybir
from concourse.bass import MemorySpace
from concourse.bass_utils import run_bass_kernel_spmd

B, T, D, H, DH = 2, 2048, 1024, 16, 64
NCORES = 8
DC = 256  # d-slice per core (4 heads, 2 head-pairs)
QSB = 512  # query superblock
NW = T // QSB  # windows (one batch per core)

f32 = mybir.dt.float32
bf16 = mybir.dt.bfloat16
AF = mybir.ActivationFunctionType

SWAP_MASK = []
for _i in range(16):
    SWAP_MASK += [2 * _i + 1, 2 * _i]

_CACHE = {}


def _build(reps=1, pipeline=True, delay_out=True, dma_norm=True, paired=True,
           pool_rope=False, norm_bf16=False, mm_tail=False, qt_bufs=4, rope_bufs=6, s_bufs=2,
           ctx_bufs=2, ost_bufs=4, p_bufs=6, out_bufs=1, proj_bufs=1, att_depth=2,
           fast_recip=False, dve_ost=False, free_ctx=False,
           ab_exp=False, ab_proj=False, ab_att=False, ab_dma=False):
    nc = bacc.Bacc("TRN2", target_bir_lowering=False, debug=False)
    xT = nc.dram_tensor("xt", [D, T], bf16, kind="ExternalInput").ap()
    wq = nc.dram_tensor("wq", [D, DC], bf16, kind="ExternalInput").ap()
    wk = nc.dram_tensor("wk", [D, DC], bf16, kind="ExternalInput").ap()
    wv = nc.dram_tensor("wv", [D, DC], bf16, kind="ExternalInput").ap()
    wo = nc.dram_tensor("wo", [DC, D], bf16, kind="ExternalInput").ap()
    ropec = nc.dram_tensor("ropec", [128, T], bf16, kind="ExternalInput").ap()
    ropes = nc.dram_tensor("ropes", [128, T], bf16, kind="ExternalInput").ap()
    mneg = nc.dram_tensor("mneg", [128, 128], f32, kind="ExternalInput").ap()
    dmask = nc.dram_tensor("dmask", [128, 2, 128], bf16, kind="ExternalInput").ap()
    vones = nc.dram_tensor("vones", [128, 64], bf16, kind="ExternalInput").ap()
    out = nc.dram_tensor("out", [T, D], bf16, kind="ExternalOutput").ap()

    nrm_dt = bf16 if norm_bf16 else f32

    with tile.TileContext(nc) as tc, ExitStack() as ctx:
        const = ctx.enter_context(tc.tile_pool(name="const", bufs=1))
        kt_pool = ctx.enter_context(tc.tile_pool(name="kt_pool", bufs=2))
        va_pool = ctx.enter_context(tc.tile_pool(name="va_pool", bufs=2))
        qt_pool = ctx.enter_context(tc.tile_pool(name="qt_pool", bufs=qt_bufs))
        rope_pool = ctx.enter_context(tc.tile_pool(name="rope_pool", bufs=rope_bufs))
        p_pool = ctx.enter_context(tc.tile_pool(name="p_pool", bufs=p_bufs))
        nrm_pool = ctx.enter_context(tc.tile_pool(name="nrm_pool", bufs=2))
        ctxn_pool = ctx.enter_context(tc.tile_pool(name="ctxn_pool", bufs=2))
        ost_pool = ctx.enter_context(tc.tile_pool(name="ost_pool", bufs=ost_bufs))

        proj_ps = ctx.enter_context(
            tc.tile_pool(name="proj_ps", bufs=proj_bufs, space=MemorySpace.PSUM)
        )
        s_ps = ctx.enter_context(
            tc.tile_pool(name="s_ps", bufs=s_bufs, space=MemorySpace.PSUM)
        )
        ctx_ps = ctx.enter_context(
            tc.tile_pool(name="ctx_ps", bufs=ctx_bufs, space=MemorySpace.PSUM)
        )
        out_ps = ctx.enter_context(
            tc.tile_pool(name="out_ps", bufs=out_bufs, space=MemorySpace.PSUM)
        )

        # ---- constants (first-use order; loads split for overlap) ----
        wq_sb = const.tile([128, 8, DC], bf16)
        wk_sb = const.tile([128, 8, DC], bf16)
        wv_sb = const.tile([128, 8, DC], bf16)
        wqr = wq.rearrange("(c p) m -> p c m", p=128)
        wkr = wk.rearrange("(c p) m -> p c m", p=128)
        wvr = wv.rearrange("(c p) m -> p c m", p=128)
        # whole x stays resident in SBUF (32KB/partition): one big
        # high-efficiency DMA for the tail, split chunks for window 0
        xa = const.tile([128, 8, T], bf16)
        xr = xT.rearrange("(c p) t -> p c t", p=128)
        for kc in range(8):
            nc.sync.dma_start(out=wq_sb[:, kc : kc + 1], in_=wqr[:, kc : kc + 1])
            nc.sync.dma_start(
                out=xa[:, kc : kc + 1, 0:QSB], in_=xr[:, kc : kc + 1, 0:QSB]
            )
        ct_sb = const.tile([128, T], bf16)
        st_sb = const.tile([128, T], bf16)
        nc.sync.dma_start(out=ct_sb[:, 0:QSB], in_=ropec[:, 0:QSB])
        nc.sync.dma_start(out=st_sb[:, 0:QSB], in_=ropes[:, 0:QSB])
        if paired:
            dmask_sb = const.tile([128, 2, 128], bf16)
            nc.sync.dma_start(out=dmask_sb, in_=dmask)
        else:
            mneg_sb = const.tile([128, 128], f32)
            nc.sync.dma_start(out=mneg_sb, in_=mneg)
        for kc in range(8):
            nc.sync.dma_start(out=wk_sb[:, kc : kc + 1], in_=wkr[:, kc : kc + 1])
        sl1 = slice(QSB, 2 * QSB)
        nc.sync.dma_start(out=xa[:, :, sl1], in_=xr[:, :, sl1])
        for kc in range(8):
            nc.sync.dma_start(out=wv_sb[:, kc : kc + 1], in_=wvr[:, kc : kc + 1])
        nc.sync.dma_start(out=ct_sb[:, sl1], in_=ropec[:, sl1])
        nc.sync.dma_start(out=st_sb[:, sl1], in_=ropes[:, sl1])
        for wi in range(2, NW):
            sl = slice(wi * QSB, (wi + 1) * QSB)
            nc.sync.dma_start(out=xa[:, :, sl], in_=xr[:, :, sl])
            nc.sync.dma_start(out=ct_sb[:, sl], in_=ropec[:, sl])
            nc.sync.dma_start(out=st_sb[:, sl], in_=ropes[:, sl])
        vones_sb = const.tile([128, 64], bf16)
        nc.sync.dma_start(out=vones_sb, in_=vones)
        wo_sb = const.tile([128, 2, D], bf16)
        nc.sync.dma_start(out=wo_sb, in_=wo.rearrange("(h p) m -> p h m", p=128))
        # preload the ACT exp/copy table under the const DMAs
        warm_sb = const.tile([1, 4], f32)
        nc.vector.memset(warm_sb, 0.0)
        nc.scalar.activation(warm_sb[0:1, 0:2], warm_sb[0:1, 2:4], AF.Exp)

        for rep in range(reps):
            KTs, VAs = {}, {}

            def prep_hp(hp, w, first):
                """Return quanta closures for (hp, w)'s projections."""
                t0 = w * QSB
                if w == 0:
                    KTs[hp] = kt_pool.tile([128, T], bf16, name=f"KT{hp}", tag="KT")
                    VAs[hp] = va_pool.tile(
                        [128, 16, 2, 65], bf16, name=f"VA{hp}", tag="VA"
                    )
                    nc.sync.dma_start(
                        out=VAs[hp][:, :, :, 64:65],
                        in_=vones[:, 0:32].rearrange("p (c h o) -> p c h o", h=2, o=1),
                    )
                KT, VA = KTs[hp], VAs[hp]
                qt = qt_pool.tile([128, QSB], bf16, name="qt")
                state = {}

                def mk_proj(wsb, key):
                    def run():
                        ps = proj_ps.tile([128, QSB], f32, tag="proj", name="ps")
                        for kc in range(4 if ab_proj else 8):
                            nc.tensor.matmul(
                                ps,
                                wsb[:, kc, 128 * hp : 128 * hp + 128],
                                xa[:, kc, t0 : t0 + QSB],
                                start=(kc == 0),
                                stop=(kc == (3 if ab_proj else 7)),
                            )
                        state[key] = ps

                    return run

                def mk_rope(key, dest_fn):
                    def run():
                        ps = state.pop(key)
                        dest = dest_fn()
                        sh = rope_pool.tile([128, QSB], f32, tag="ropet", name="sh")
                        nc.vector.stream_shuffle(sh, ps, SWAP_MASK)
                        m1 = rope_pool.tile([128, QSB], f32, tag="ropet", name="m1")
                        nc.vector.tensor_mul(m1, ps, ct_sb[:, t0 : t0 + QSB])
                        m2 = rope_pool.tile([128, QSB], f32, tag="ropet", name="m2")
                        if pool_rope:
                            nc.gpsimd.tensor_mul(m2, sh, st_sb[:, t0 : t0 + QSB])
                        else:
                            nc.vector.tensor_mul(m2, sh, st_sb[:, t0 : t0 + QSB])
                        nc.gpsimd.tensor_add(dest, m1, m2)

                    return run

                quanta = [
                    mk_proj(wq_sb, "q"),
                    mk_rope("q", lambda: qt),
                    mk_proj(wk_sb, "k"),
                    mk_rope("k", lambda: KT[:, t0 : t0 + QSB]),
                ]
                return qt, quanta

            def mk_vproj(tb):
                """V computed directly in [keys, dims] layout (x-chunk
                stationary, Wv moving) — no transpose needed."""

                def run():
                    ps = proj_ps.tile([128, DC], f32, tag="proj", name="vps")
                    blk = tb * 128
                    for kc in range(8):
                        nc.tensor.matmul(
                            ps,
                            xa[:, kc, blk : blk + 128],
                            wv_sb[:, kc],
                            start=(kc == 0),
                            stop=(kc == 7),
                        )
                    for hp in range(2):
                        nc.vector.tensor_copy(
                            VAs[hp][:, tb, :, 0:64],
                            ps[:, 128 * hp : 128 * hp + 128].rearrange(
                                "p (h d) -> p h d", h=2
                            ),
                        )

                return run

            def prep_window(w, first):
                """Return per-hp (qt, quanta) plus shared V-projection quanta."""
                out = []
                for hp in range(2):
                    out.append(prep_hp(hp, w, first))
                vq = [mk_vproj(4 * w + i) for i in range(4)]
                return out, vq

            def outproj_quanta(ctxns, t0, ps_pool=None):
                """Closures per (ts, eh) matmul+copy; one 2KB-row DMA per ts."""
                osts = {}
                pool = ps_pool if ps_pool is not None else out_ps
                ops_tag = "s" if ps_pool is not None else "ops"

                def mk(ts, eh):
                    def run():
                        if eh == 0:
                            osts[ts] = ost_pool.tile(
                                [128, 2, 512], bf16, name="ost", tag="ost"
                            )
                        ops = pool.tile([128, 512], f32, name="ops", tag=ops_tag)
                        for hp in range(2):
                            nc.tensor.matmul(
                                ops,
                                ctxns[hp][:, 128 * ts : 128 * ts + 128],
                                wo_sb[:, hp, 512 * eh : 512 * eh + 512],
                                start=(hp == 0),
                                stop=(hp == 1),
                            )
                        if eh == 0:
                            nc.vector.tensor_copy(osts[ts][:, 0], ops)
                        else:
                            if dve_ost:
                                nc.vector.tensor_copy(osts[ts][:, 1], ops)
                            else:
                                nc.scalar.copy(osts[ts][:, 1], ops)
                            if not ab_dma:
                                nc.sync.dma_start(
                                    out=out[t0 + 128 * ts : t0 + 128 * ts + 128, :],
                                    in_=osts.pop(ts).rearrange("p a n -> p (a n)"),
                                )
                            else:
                                osts.pop(ts)

                    return run

                return [mk(ts, eh) for ts in range(4) for eh in range(2)]

            seq = [(w, hp) for w in range(NW) for hp in range(2)]
            qts = {}
            prep0, vq0 = prep_window(0, first=(rep == 0))
            for hp in range(2):
                qt0, quanta0 = prep0[hp]
                qts[(0, hp)] = qt0
                if hp == 0:
                    for q in quanta0:
                        q()
                else:
                    nq_carry = quanta0
            for q in vq0:
                q()
            pending = None

            for i, (w, hp) in enumerate(seq):
                t0 = w * QSB
                KT, VA = KTs[hp], VAs[hp]
                qt = qts.pop((w, hp))

                # quanta to interleave: delayed out-projection, remainder of
                # this window's other hp, then next window's projections
                nq = []
                if delay_out and pending is not None:
                    nq += outproj_quanta(*pending)
                    pending = None
                nq += nq_carry
                nq_carry = []
                if hp == 1 and w + 1 < NW:
                    prepn, vqn = prep_window(w + 1, first=False)
                    qtn0, quanta_n0 = prepn[0]
                    qtn1, quanta_n1 = prepn[1]
                    qts[(w + 1, 0)] = qtn0
                    qts[(w + 1, 1)] = qtn1
                    nq += quanta_n0 + vqn
                    nq_carry = quanta_n1
                if not pipeline:
                    for q in nq:
                        q()
                    nq = []

                nkb = 4 * w + 4
                cps = [
                    ctx_ps.tile([65, QSB], f32, tag="ctx", name=f"cps{h}")
                    for h in range(2)
                ]
                emitted = 0
                kbs = list(range(nkb))[:: (2 if ab_att else 1)]
                last_kb = kbs[-1]

                def emit_scores(kb):
                    o = kb - 4 * w
                    col0 = 128 * o if o > 0 else 0
                    ncols = QSB - col0
                    excols = ncols // 2 if ab_exp else ncols
                    if paired:
                        spp = s_ps.tile([128, 2, QSB], f32, tag="s", name="spp")
                        for h in range(2):
                            nc.tensor.matmul(
                                spp[:, h, :ncols],
                                KT[64 * h : 64 * h + 64, 128 * kb : 128 * kb + 128],
                                qt[64 * h : 64 * h + 64, col0:QSB],
                                start=True,
                                stop=True,
                            )
                        ptp = p_pool.tile([128, 2, QSB], bf16, tag="pt", name="pt")
                        nc.scalar.activation(
                            ptp[:, :, :excols], spp[:, :, :excols], AF.Exp, scale=0.125
                        )
                        if o >= 0:
                            nc.vector.tensor_mul(
                                ptp[:, :, 0:128], ptp[:, :, 0:128], dmask_sb
                            )
                        return [ptp[:, h] for h in range(2)]
                    pts = []
                    for h in range(2):
                        sps = s_ps.tile([128, QSB], f32, tag="s", name="sps")
                        nc.tensor.matmul(
                            sps[:, :ncols],
                            KT[64 * h : 64 * h + 64, 128 * kb : 128 * kb + 128],
                            qt[64 * h : 64 * h + 64, col0:QSB],
                            start=True,
                            stop=True,
                        )
                        if o >= 0:
                            nc.vector.tensor_add(sps[:, 0:128], sps[:, 0:128], mneg_sb)
                        pt = p_pool.tile([128, QSB], bf16, tag="pt", name="pt")
                        nc.scalar.activation(
                            pt[:, :excols], sps[:, :excols], AF.Exp, scale=0.125
                        )
                        pts.append(pt)
                    return pts

                def emit_ctx(kb, pts):
                    o = kb - 4 * w
                    col0 = 128 * o if o > 0 else 0
                    ncols = QSB - col0
                    for h in range(2):
                        nc.tensor.matmul(
                            cps[h][:, col0:QSB],
                            VA[:, kb, h],
                            pts[h][:, :ncols],
                            start=(kb == 0),
                            stop=(kb == last_kb),
                        )

                # software-pipeline the block loop att_depth deep: block kb's
                # ctx waits on its exp, so emit the next block(s)' scores (and
                # interleave quanta) in between
                inflight = []
                for idx, kb in enumerate(kbs):
                    inflight.append((kb, emit_scores(kb)))
                    want = (idx + 1) * len(nq) // len(kbs)
                    while emitted < want:
                        nq[emitted]()
                        emitted += 1
                    if len(inflight) >= att_depth:
                        ckb, cpts = inflight.pop(0)
                        emit_ctx(ckb, cpts)
                for ckb, cpts in inflight:
                    emit_ctx(ckb, cpts)
                while emitted < len(nq):
                    nq[emitted]()
                    emitted += 1

                # normalize this (w, hp)
                ctxn = ctxn_pool.tile([128, QSB], bf16, name=f"ctxn{hp}", tag="ctxn")
                from contextlib import nullcontext

                tail = mm_tail and w == NW - 1
                for h in range(2):
                    if free_ctx:
                        # evacuate the ctx accumulator to SBUF right away so
                        # the PSUM bank frees for the next (w, hp)'s attention
                        # instead of being held through the normalize chain
                        cc = nrm_pool.tile([65, QSB], f32, tag="cc", name="cc")
                        nc.scalar.copy(cc, cps[h])
                        src = cc
                    else:
                        src = cps[h]
                    rdt = bf16 if tail else nrm_dt
                    rc = nrm_pool.tile([65, QSB], rdt, tag="rc", name="rc")
                    if fast_recip and rdt == f32:
                        nc.vector.reciprocal_approx_fast(rc[64:65, :], src[64:65, :])
                    else:
                        with (
                            nc.allow_low_precision(reason="bf16 softmax denominator")
                            if (norm_bf16 or tail)
                            else nullcontext()
                        ):
                            nc.vector.reciprocal(rc[64:65, :], src[64:65, :])
                    if tail:
                        # proj_ps banks are idle in the last window: broadcast
                        # 1/den across partitions with a rank-1 matmul (short
                        # latency, no DMA/Pool on the critical tail)
                        bc = proj_ps.tile([64, QSB], f32, tag="proj", name="bcmm")
                        nc.tensor.matmul(
                            bc, vones_sb[64:65, :], rc[64:65, :], start=True, stop=True
                        )
                    elif dma_norm:
                        bc = nrm_pool.tile([64, QSB], nrm_dt, tag="bc", name="bc")
                        rc0 = nrm_pool.tile([1, QSB], nrm_dt, tag="rc0", name="rc0")
                        nc.sync.dma_start(out=rc0, in_=rc[64:65, :])
                        nc.gpsimd.partition_broadcast(bc, rc0, channels=64)
                    else:
                        bc = nrm_pool.tile([64, QSB], nrm_dt, tag="bc", name="bc")
                        nc.gpsimd.partition_broadcast(bc, rc[64:65, :], channels=64)
                    if h == 0:
                        nc.vector.tensor_mul(ctxn[0:64, :], src[0:64, :], bc)
                    else:
                        cn1 = nrm_pool.tile([64, QSB], bf16, tag="cn1", name="cn1")
                        nc.vector.tensor_mul(cn1, src[0:64, :], bc)
                        if not ab_dma:
                            nc.sync.dma_start(out=ctxn[64:128, :], in_=cn1)
                if hp == 0:
                    ctxn0 = ctxn
                else:
                    if delay_out:
                        pending = ((ctxn0, ctxn), t0)
                    else:
                        for q in outproj_quanta((ctxn0, ctxn), t0):
                            q()

            if pending is not None:
                # attention is over: scores banks are free, use them so the
                # tail out-projection isn't serialized on out_ps reuse
                for q in outproj_quanta(*pending, ps_pool=s_ps):
                    q()

    nc.compile()
    return nc


def _host_inputs(x, Wq, Wk, Wv, Wo):
    import ml_dtypes

    bfloat16 = ml_dtypes.bfloat16

    pos = np.arange(T, dtype=np.float64)
    inv_freq = np.power(10000.0, -2.0 * np.arange(0, DH, 2) / DH)  # (32,)
    freqs = pos[:, None] * inv_freq[None, :]  # (T, 32)
    cos = np.cos(freqs)
    sin = np.sin(freqs)
    ct = np.empty((128, T), np.float32)
    st = np.empty((128, T), np.float32)
    for p in range(128):
        i = (p % DH) // 2
        ct[p] = cos[:, i]
        st[p] = sin[:, i] * (-1.0 if p % 2 == 0 else 1.0)
    ct = ct.astype(bfloat16)
    st = st.astype(bfloat16)

    pp, cc = np.meshgrid(np.arange(128), np.arange(128), indexing="ij")
    mneg = np.where(pp <= cc, 0.0, -1e9).astype(np.float32)
    dmask1 = np.where(pp <= cc, 1.0, 0.0).astype(bfloat16)
    dmask = np.stack([dmask1, dmask1], axis=1)  # [128, 2, 128]
    vones = np.ones((128, 64), bfloat16)

    xb = [np.ascontiguousarray(x[b].T).astype(bfloat16) for b in range(B)]
    per_core = []
    for c in range(NCORES):
        b, hg = c // 4, c % 4
        sl = slice(hg * DC, (hg + 1) * DC)
        per_core.append(
            {
                "xt": xb[b],
                "wq": np.ascontiguousarray(Wq[:, sl]).astype(bfloat16),
                "wk": np.ascontiguousarray(Wk[:, sl]).astype(bfloat16),
                "wv": np.ascontiguousarray(Wv[:, sl]).astype(bfloat16),
                "wo": np.ascontiguousarray(Wo[sl, :]).astype(bfloat16),
                "ropec": ct,
                "ropes": st,
                "mneg": mneg,
                "dmask": dmask,
                "vones": vones,
            }
        )
    return per_core


def kernel(x, Wq, Wk, Wv, Wo, bo):
    x = np.asarray(x, np.float32)
    Wq = np.asarray(Wq, np.float32)
    Wk = np.asarray(Wk, np.float32)
    Wv = np.asarray(Wv, np.float32)
    Wo = np.asarray(Wo, np.float32)
    bo = np.asarray(bo, np.float32)

    if "nc" not in _CACHE:
        _CACHE["nc"] = _build()
    nc = _CACHE["nc"]

    in_maps = _host_inputs(x, Wq, Wk, Wv, Wo)
    res = run_bass_kernel_spmd(nc, in_maps, list(range(NCORES)))
    outs = []
    for b in range(B):
        acc = res.results[4 * b]["out"].astype(np.float64)
        for hg in range(1, 4):
            acc += res.results[4 * b + hg]["out"].astype(np.float64)
        outs.append(acc + bo.astype(np.float64))
    return np.stack(outs).astype(np.float32)
